# revision 33
# baseline (speedup 1.0000x reference)
"""Trainium2 Bass kernel for nn_EvMLP (segment_reduce EvNorm + invariant MLP).

Self-contained: hardcodes shapes/sharding. Accepts FULL inputs, returns FULL
output; shards the node dim N across 8 NeuronCores (pure data parallel).

v2 design (bf16 edition):
  - bf16 HBM I/O both directions (host converts; halves DMA traffic)
  - x10 loaded pre-transposed via DMA-transpose (HWDGE xbar) -> no PE
    transposes / ACT copies for the MLP input
  - single Newton-Raphson step for all rsqrts (quake seed on ACT)
  - W12 = w1p @ w2p host-side fusion: H1 only feeds the LN1 variance;
    H2 computed directly from x1T (kills the h1 PSUM->SBUF copy)
  - b3 bias added via rank-1 PE matmul into PSUM; out copy is plain ACT
  - bf16 matmuls (PE 2x), bf16 elementwise where DVE 2x modes apply
  - row order is linear "(b p)" so the DMA-transposed x10T and the
    eq-path tiles agree on row placement
"""
import sys

sys.path.insert(0, "/opt/trn_rl_repo")

import numpy as np

import concourse.bass as bass
import concourse.bacc as bacc
import concourse.tile as tile
from concourse import mybir
from concourse.bass_utils import run_bass_kernel_spmd

f32 = mybir.dt.float32
i32 = mybir.dt.int32
bf16 = mybir.dt.bfloat16

# ---------------------------------------------------------------- constants --
N = 100000
DIM = 592
N_INV = 128
N_EQ_CH = 112
N_EQ = 464
EPS = 1e-5
N_CORES = 8
BLOCKS_PER_CORE = 98                      # 98*128 = 12544 rows/core
ROWS_PER_CORE = BLOCKS_PER_CORE * 128
NPAD = N_CORES * ROWS_PER_CORE            # 100352
MACROS = [8] * 12 + [2]                   # blocks per macro-tile (sum 98)
CHUNK_BLOCKS = 4                          # rows per MLP chunk = 512
MAGIC = 0x5F3759DF
MAGICF = float(MAGIC)

# knob: eq column split for the square: [0:KA] ACT, [KA:KB] DVE, [KB:] GPSIMD
SQ_ACT_COLS = 192
SQ_DVE_COLS = 192

# segment groups: (n_channels, width, eq column offset, channel offset)
SEGS = [(64, 3, 0, 0), (32, 5, 192, 64), (16, 7, 352, 96)]

_EXPECTED_REP = np.concatenate(
    [np.repeat(np.arange(m) + off, 2 * l + 1)
     for l, (m, off) in enumerate([(128, 0), (64, 128), (32, 192), (16, 224)])]
)

# ------------------------------------------------------------- custom DVE op --
from concourse.dve_spec import Spec, Src0, Src1, C0, C1, C2, lower
from concourse.dve_uop import DveOpSpec
import concourse.dve_ops as dve_ops
from concourse.dve_ops import DveOp

# Newton rsqrt step: out = y*(C1 - C0*((v+C2)*y*y));  in0=v, in1=y
_nr_body = Src1 * (C1 - ((Src0 + C2) * (Src1 * Src1)) * C0)


def _nr_ref(in0, in1, s0, s1, imm2):
    y = in1.astype(np.float32)
    v = in0.astype(np.float32)
    return (y * (np.float32(s1) - ((v + np.float32(imm2)) * y * y) * np.float32(s0))
            ).astype(np.float32)


def _register(name, spec):
    if name in dve_ops._SUB_OPCODE_FOR_NAME:
        for op in dve_ops.OPS:
            if op.name == name:
                return op
    shas = {}
    row = 1 + len(dve_ops.OPS)
    for ver in ("v3", "v4"):
        s = DveOpSpec(name=name, opcode=row, uops=lower(spec, ver=ver), rd1_en=True)
        shas[ver] = s.sha(ver)
    op = DveOp(name, spec, subdim=False, uops_sha=shas)
    dve_ops.OPS.append(op)
    dve_ops._SUB_OPCODE_FOR_NAME[name] = row
    dve_ops.CUSTOM_DVE_SPECS[name] = spec
    return op


RSQRT_NR = _register("ANT_RSQRT_NR2", Spec(body=_nr_body, reference=_nr_ref))


def _make_mulsub1():
    from concourse.dve_spec import One
    return _register(
        "ANT_MUL_SUB1",
        Spec(
            body=(Src0 * Src1) - One,
            reference=lambda in0, in1, s0, s1, imm2: (
                in0.astype(np.float32) * in1 - np.float32(1.0)
            ).astype(np.float32),
        ),
    )


MUL_SUB1 = _make_mulsub1()


# ------------------------------------------------------------ kernel builder --
def _build_nc():
    nc = bacc.Bacc()

    x = nc.dram_tensor("x", [ROWS_PER_CORE, DIM], bf16, kind="ExternalInput")
    out = nc.dram_tensor("out", [ROWS_PER_CORE, DIM], bf16, kind="ExternalOutput")
    w1a_d = nc.dram_tensor("w1a", [128, 128], bf16, kind="ExternalInput")
    w1b_d = nc.dram_tensor("w1b", [112, 128], bf16, kind="ExternalInput")
    w12a_d = nc.dram_tensor("w12a", [128, 128], bf16, kind="ExternalInput")
    w12b_d = nc.dram_tensor("w12b", [112, 128], bf16, kind="ExternalInput")
    w3_d = nc.dram_tensor("w3p", [128, 128], bf16, kind="ExternalInput")
    cmat_d = nc.dram_tensor("cmat", [128, 128], bf16, kind="ExternalInput")
    onesd_d = nc.dram_tensor("onesd", [128, 128], bf16, kind="ExternalInput")
    ident_d = nc.dram_tensor("ident", [128, 128], bf16, kind="ExternalInput")
    b2_d = nc.dram_tensor("b2c", [128, 1], f32, kind="ExternalInput")
    ones1_d = nc.dram_tensor("ones1", [1, 128], bf16, kind="ExternalInput")
    b3r_d = nc.dram_tensor("b3rep", [1, 1024], bf16, kind="ExternalInput")

    # extra float consts used as activation bias (register like Bass.__init__)
    for _v in (MAGICF, float(EPS), 1.0):
        _t = nc.alloc_sbuf_tensor(f"const-f32-{_v}", [128, 1], f32)
        nc.gpsimd.memset(_t.ap(), _v)
        nc.const_aps.aps[(f32, _v)] = _t.ap()
    nc.all_engine_barrier()

    AF = mybir.ActivationFunctionType
    ALU = mybir.AluOpType
    AX = mybir.AxisListType

    from contextlib import ExitStack

    with tile.TileContext(nc) as tc:
        with ExitStack() as ctx:
            wpool = ctx.enter_context(tc.tile_pool(name="w", bufs=1))
            xpool = ctx.enter_context(tc.tile_pool(name="xp", bufs=4))
            opool = ctx.enter_context(tc.tile_pool(name="op", bufs=3))
            epool = ctx.enter_context(tc.tile_pool(name="ep", bufs=2))
            spool = ctx.enter_context(tc.tile_pool(name="sp", bufs=3))
            cpool = ctx.enter_context(tc.tile_pool(name="cp", bufs=3))
            ps_tp = ctx.enter_context(tc.tile_pool(name="ptp", bufs=2, space="PSUM"))
            ps_mm = ctx.enter_context(tc.tile_pool(name="pmm", bufs=4, space="PSUM"))
            ps_q = ctx.enter_context(tc.tile_pool(name="pq", bufs=2, space="PSUM"))

            def wtile(name, dram, shape, dtype):
                t = wpool.tile(shape, dtype, tag=name)
                nc.sync.dma_start(out=t, in_=dram[:, :])
                return t

            w1a = wtile("w1a", w1a_d, [128, 128], bf16)
            w1b = wtile("w1b", w1b_d, [112, 128], bf16)
            w12a = wtile("w12a", w12a_d, [128, 128], bf16)
            w12b = wtile("w12b", w12b_d, [112, 128], bf16)
            w3p = wtile("w3p", w3_d, [128, 128], bf16)
            cmat = wtile("cmat", cmat_d, [128, 128], bf16)
            onesd = wtile("onesd", onesd_d, [128, 128], bf16)
            ident = wtile("ident", ident_d, [128, 128], bf16)
            b2c = wtile("b2c", b2_d, [128, 1], f32)
            ones1 = wtile("ones1", ones1_d, [1, 128], bf16)
            b3rep = wtile("b3rep", b3r_d, [1, 1024], bf16)

            flat3 = lambda ap: ap.rearrange("p a b -> p (a b)")

            def mm512(out_ap, lhsT, rhs, start, stop, skip=False):
                # ISA caps the moving operand at 512 elements per matmul
                F = rhs.shape[-1]
                for f0 in range(0, F, 512):
                    f1 = min(f0 + 512, F)
                    nc.tensor.matmul(
                        out_ap[:, f0:f1], lhsT, rhs[:, f0:f1],
                        start=start, stop=stop, skip_group_check=skip,
                    )

            def issue_load(row0, nb):
                """Prefetch macro tiles (issued one iteration early)."""
                R_rows = nb * 128
                # linear row order: partition p, block b <-> row b*128 + p
                xev = x[row0 : row0 + R_rows, N_INV:DIM].rearrange(
                    "(b p) d -> p b d", p=128
                )
                x10v = x[row0 : row0 + R_rows, 0:N_INV]
                ov = out[row0 : row0 + R_rows, :].rearrange("(b p) d -> p b d", p=128)

                Xe = xpool.tile([128, nb, N_EQ], bf16, tag="Xe")
                nc.sync.dma_start(out=Xe, in_=xev)
                X10T = xpool.tile([128, R_rows], bf16, tag="X10T")
                nc.sync.dma_start(out=X10T, in_=x10v, transpose=True)
                return dict(nb=nb, ov=ov, Xe=Xe, X10T=X10T)

            def issue_eq(t):
                """eq chain through x11 (inputs prefetched last iteration)."""
                nb, Xe = t["nb"], t["Xe"]
                # ---- eq path (rows on partitions) ----
                # squares written T-MAJOR per segment group so the segment
                # sums become contiguous bf16 TT adds (DVE 2x_1p mode);
                # TensorReduce has no fast modes so this beats reduce_sum.
                eq2 = epool.tile([128, nb, N_EQ], bf16, tag="eq2")
                # group 1 (64ch x 3) squared on ACT; groups 2,3 on GPSIMD
                for ei, (nch, w, eqoff, choff) in enumerate(SEGS):
                    eng = nc.scalar if ei == 0 else None
                    ov_ = eq2[:, :, eqoff : eqoff + nch * w].rearrange(
                        "p b (t c) -> p b c t", c=nch
                    )
                    iv_ = Xe[:, :, eqoff : eqoff + nch * w].rearrange(
                        "p b (c t) -> p b c t", t=w
                    )
                    if ei == 0:
                        nc.scalar.activation(out=ov_, in_=iv_, func=AF.Square)
                    else:
                        nc.gpsimd.tensor_tensor(out=ov_, in0=iv_, in1=iv_,
                                                op=ALU.mult)

                sumsq = spool.tile([128, nb, N_EQ_CH], bf16, tag="sumsq")
                def lv(eqoff, nch, t):
                    return eq2[:, :, eqoff + t * nch : eqoff + (t + 1) * nch]
                def tadd(o, a, b):
                    nc.vector.tensor_tensor(out=o, in0=a, in1=b, op=ALU.add)
                def scr(n, tag):
                    return spool.tile([128, nb, n], bf16, tag=tag, name=tag)
                # group 1: 3-wide
                s01 = scr(64, "g1a"); tadd(s01, lv(0, 64, 0), lv(0, 64, 1))
                tadd(sumsq[:, :, 0:64], s01, lv(0, 64, 2))
                # group 2: 5-wide
                u = scr(32, "g2a"); tadd(u, lv(192, 32, 0), lv(192, 32, 1))
                v = scr(32, "g2b"); tadd(v, lv(192, 32, 2), lv(192, 32, 3))
                uv = scr(32, "g2c"); tadd(uv, u, v)
                tadd(sumsq[:, :, 64:96], uv, lv(192, 32, 4))
                # group 3: 7-wide
                a3 = scr(16, "g3a"); tadd(a3, lv(352, 16, 0), lv(352, 16, 1))
                b3_ = scr(16, "g3b"); tadd(b3_, lv(352, 16, 2), lv(352, 16, 3))
                c3 = scr(16, "g3c"); tadd(c3, lv(352, 16, 4), lv(352, 16, 5))
                d3 = scr(16, "g3d"); tadd(d3, a3, b3_)
                e3 = scr(16, "g3e"); tadd(e3, c3, lv(352, 16, 6))
                tadd(sumsq[:, :, 96:112], d3, e3)

                # s1 = sumsq + 1 (ACT); seed from its bits (ACT); 1 NR (DVE)
                s1 = spool.tile([128, nb, N_EQ_CH], f32, tag="s1")
                nc.scalar.activation(out=s1, in_=sumsq, func=AF.Identity, bias=1.0)
                seedb = spool.tile([128, nb, N_EQ_CH], i32, tag="seedb")
                nc.scalar.activation(
                    out=seedb, in_=s1.bitcast(i32), func=AF.Identity,
                    scale=-0.5, bias=MAGICF,
                )
                r = spool.tile([128, nb, N_EQ_CH], f32, tag="r")
                nc.vector._custom_dve(
                    RSQRT_NR, out=flat3(r), in0=flat3(s1),
                    in1=flat3(seedb.bitcast(f32)), s0=0.5, s1=1.5, imm2=0.0,
                )

                # x11 = s1 * r - 1  (= sqrt(s1) - 1)
                x11 = spool.tile([128, nb, N_EQ_CH], bf16, tag="x11")
                nc.vector._custom_dve(
                    MUL_SUB1, out=flat3(x11), in0=flat3(s1), in1=flat3(r),
                    s0=0.0, s1=0.0, imm2=0.0,
                )
                t["r"] = r
                t["x11"] = x11

            def issue_T(t):
                """Transposes of x11 + copy (inputs one iteration old)."""
                nb = t["nb"]
                x11 = t["x11"]
                chunks = []
                for cb0 in range(0, nb, CHUNK_BLOCKS):
                    cnb = min(CHUNK_BLOCKS, nb - cb0)
                    chunks.append((cb0, cnb, cnb * 128))
                t["chunks"] = chunks
                st = {}
                for ci, (cb0, cnb, R) in enumerate(chunks):
                    TPb = ps_tp.tile([N_EQ_CH, R], bf16, tag="tp")
                    for j in range(cnb):
                        nc.tensor.transpose(
                            TPb[:, j * 128 : (j + 1) * 128],
                            x11[:, cb0 + j, :], ident,
                        )
                    st[ci] = TPb
                xt = {}
                for ci, (cb0, cnb, R) in enumerate(chunks):
                    x11T = cpool.tile([N_EQ_CH, R], bf16, tag="x11T")
                    nc.scalar.activation(out=x11T, in_=st[ci], func=AF.Identity)
                    xt[ci] = x11T
                t["xt"] = xt

            def issue_L1(t):
                """x2 (GP), M1 + variance square."""
                nb = t["nb"]
                Xe, X10T, r = t["Xe"], t["X10T"], t["r"]
                chunks, xt = t["chunks"], t["xt"]
                O = opool.tile([128, nb, DIM], bf16, tag="O")
                t["O"] = O

                for (nch, w, eqoff, choff) in SEGS:
                    rbc = (
                        r[:, :, choff : choff + nch]
                        .unsqueeze(-1)
                        .broadcast_to((128, nb, nch, w))
                    )
                    nc.gpsimd.tensor_tensor(
                        out=O[:, :, N_INV + eqoff : N_INV + eqoff + nch * w].rearrange(
                            "p b (c t) -> p b c t", t=w
                        ),
                        in0=Xe[:, :, eqoff : eqoff + nch * w].rearrange(
                            "p b (c t) -> p b c t", t=w
                        ),
                        in1=rbc,
                        op=ALU.mult,
                    )

                sq1s = {}
                for ci, (cb0, cnb, R) in enumerate(chunks):
                    x10c = X10T[:, cb0 * 128 : cb0 * 128 + R]
                    H1 = ps_mm.tile([128, R], f32, tag="mm")
                    mm512(H1, w1a, x10c, True, False)
                    mm512(H1, w1b, xt[ci], False, True)
                    sq1 = cpool.tile([128, R], bf16, tag="sq1")
                    nc.scalar.activation(out=sq1, in_=H1, func=AF.Square)
                    sq1s[ci] = sq1
                t["sq1s"] = sq1s
                h2s = {}
                for ci, (cb0, cnb, R) in enumerate(chunks):
                    x10c = X10T[:, cb0 * 128 : cb0 * 128 + R]
                    H2 = ps_mm.tile([128, R], f32, tag="mm")
                    mm512(H2, w12a, x10c, True, False)
                    mm512(H2, w12b, xt[ci], False, True)
                    h2s[ci] = H2
                t["h2s"] = h2s

            def issue_L2(t):
                """LN1 stats+apply, silu, center, LN2 stats+apply."""
                nb = t["nb"]
                chunks, sq1s = t["chunks"], t["sq1s"]

                ln1 = {}
                for ci, (cb0, cnb, R) in enumerate(chunks):
                    Q1 = ps_q.tile([128, R], f32, tag="qq")
                    mm512(Q1, onesd, sq1s[ci], True, True)
                    sd1 = cpool.tile([128, R], i32, tag="sd1")
                    nc.scalar.activation(out=sd1, in_=Q1.bitcast(i32),
                                         func=AF.Identity, scale=-0.5, bias=MAGICF)
                    rstd1 = cpool.tile([128, R], f32, tag="rstd1")
                    nc.vector._custom_dve(
                        RSQRT_NR, out=rstd1, in0=Q1, in1=sd1.bitcast(f32),
                        s0=0.5, s1=1.5, imm2=float(EPS),
                    )
                    ln1[ci] = rstd1

                ln2 = {}
                h2s = t["h2s"]
                for ci, (cb0, cnb, R) in enumerate(chunks):
                    H2 = h2s[ci]
                    av = cpool.tile([128, R], bf16, tag="av")
                    nc.vector.tensor_mul(av, H2, ln1[ci])
                    a2 = cpool.tile([128, R], bf16, tag="a2")
                    nc.scalar.activation(out=a2, in_=av, func=AF.Silu, bias=b2c)
                    AC = ps_mm.tile([128, R], f32, tag="mm")
                    mm512(AC, cmat, a2, True, True)
                    sq2 = cpool.tile([128, R], bf16, tag="sq2")
                    nc.scalar.activation(out=sq2, in_=AC, func=AF.Square)
                    Q2 = ps_q.tile([128, R], f32, tag="qq")
                    mm512(Q2, onesd, sq2, True, True)
                    sd2 = cpool.tile([128, R], i32, tag="sd2")
                    nc.scalar.activation(out=sd2, in_=Q2.bitcast(i32),
                                         func=AF.Identity, scale=-0.5, bias=MAGICF)
                    rstd2 = cpool.tile([128, R], f32, tag="rstd2")
                    nc.vector._custom_dve(
                        RSQRT_NR, out=rstd2, in0=Q2, in1=sd2.bitcast(f32),
                        s0=0.5, s1=1.5, imm2=float(EPS),
                    )
                    hn2 = cpool.tile([128, R], bf16, tag="hn2")
                    nc.vector.tensor_mul(hn2, AC, rstd2)
                    ln2[ci] = hn2
                t["ln2"] = ln2

            def issue_L3(t):
                """M3 (+bias), out copy, store."""
                nb, ov, O = t["nb"], t["ov"], t["O"]
                chunks, ln2 = t["chunks"], t["ln2"]
                for ci, (cb0, cnb, R) in enumerate(chunks):
                    hn2 = ln2[ci]
                    H3n = ps_mm.tile([128, R], f32, tag="mm")
                    mm512(H3n, ones1, b3rep[:, 0:R], True, False, skip=True)
                    for j in range(cnb):
                        nc.tensor.matmul(
                            H3n[:, j * 128 : (j + 1) * 128],
                            hn2[:, j * 128 : (j + 1) * 128], w3p,
                            start=False, stop=True,
                            skip_group_check=True,
                        )
                    nc.scalar.activation(
                        out=O[:, cb0 : cb0 + cnb, 0:N_INV],
                        in_=H3n.rearrange("p (b j) -> p b j", j=128),
                        func=AF.Identity,
                    )
                nc.sync.dma_start(out=ov, in_=O)

            # ---- two-stage skewed software pipeline (v7b order) ----
            offs = []
            row0 = 0
            for nb in MACROS:
                offs.append((row0, nb))
                row0 += nb * 128
            states = {}
            nmac = len(offs)
            states[0] = issue_load(*offs[0])
            for i in range(nmac + 2):
                if i + 1 < nmac:
                    states[i + 1] = issue_load(*offs[i + 1])
                if i < nmac:
                    issue_eq(states[i])
                if 0 <= i - 1 < nmac:
                    issue_T(states[i - 1])
                    issue_L1(states[i - 1])
                if 0 <= i - 2 < nmac:
                    issue_L2(states[i - 2])
                    issue_L3(states[i - 2])
                    del states[i - 2]

    nc.finalize()
    return nc


_NC_CACHE = {}


def _get_nc():
    if "nc" not in _NC_CACHE:
        _NC_CACHE["nc"] = _build_nc()
    return _NC_CACHE["nc"]


# --------------------------------------------------------------- host driver --
def _bf16(a):
    import ml_dtypes
    return np.asarray(a).astype(ml_dtypes.bfloat16)


def _prep_weights(w1, g1, beta1, w2, b2, g2, beta2, w3, b3):
    C = np.eye(128, dtype=np.float64) - 1.0 / 128.0
    w1p = w1.astype(np.float64) @ C                       # [240,128]
    w2p = (g1.astype(np.float64)[:, None] * w2.astype(np.float64))
    b2c = beta1.astype(np.float64) @ w2.astype(np.float64) + b2.astype(np.float64)
    W12 = w1p @ w2p
    w3p = (g2.astype(np.float64)[:, None] * w3.astype(np.float64))
    b3c = beta2.astype(np.float64) @ w3.astype(np.float64) + b3.astype(np.float64)
    return {
        "w1a": _bf16(w1p[0:128]),
        "w1b": _bf16(w1p[128:240]),
        "w12a": _bf16(W12[0:128]),
        "w12b": _bf16(W12[128:240]),
        "w3p": _bf16(w3p),
        "cmat": _bf16(C),
        "onesd": _bf16(np.full((128, 128), 1.0 / 128.0)),
        "ident": _bf16(np.eye(128)),
        "b2c": b2c.astype(np.float32).reshape(128, 1),
        "ones1": _bf16(np.ones((1, 128))),
        "b3rep": _bf16(np.tile(b3c.reshape(1, 128), (1, 8))),
    }


def _np_reference(ten, w1, g1, beta1, w2, b2, g2, beta2, w3, b3):
    """Pure-numpy fallback (used only if rep_layout is unexpected)."""
    x10 = ten[:, :N_INV]
    eq = ten[:, N_INV:]
    sumsq = np.zeros((ten.shape[0], N_EQ_CH), np.float32)
    for (nch, w, eqoff, choff) in SEGS:
        sumsq[:, choff:choff + nch] = (
            (eq[:, eqoff:eqoff + nch * w].reshape(-1, nch, w) ** 2).sum(-1)
        )
    d = np.sqrt(sumsq + 1.0)
    x11 = d - 1.0
    x1 = np.concatenate([x10, x11], 1)
    seg = np.concatenate([np.repeat(np.arange(nch) + choff, w)
                          for (nch, w, eqoff, choff) in SEGS])
    x2 = eq / d[:, seg]

    def ln(h, g, b):
        mu = h.mean(-1, keepdims=True)
        var = h.var(-1, keepdims=True)
        return (h - mu) / np.sqrt(var + EPS) * g + b

    h = x1 @ w1
    h = ln(h, g1, beta1)
    h = h @ w2 + b2
    h = h * (1.0 / (1.0 + np.exp(-h)))
    h = ln(h, g2, beta2)
    h = h @ w3 + b3
    return np.concatenate([h, x2], 1).astype(np.float32)


def kernel(ten, rep_layout, w1, g1, beta1, w2, b2, g2, beta2, w3, b3):
    ten = np.asarray(ten, dtype=np.float32)
    args = [np.asarray(a) for a in (w1, g1, beta1, w2, b2, g2, beta2, w3, b3)]
    w1, g1, beta1, w2, b2, g2, beta2, w3, b3 = [a.astype(np.float32) for a in args]

    if not np.array_equal(np.asarray(rep_layout).astype(np.int64), _EXPECTED_REP):
        return _np_reference(ten, w1, g1, beta1, w2, b2, g2, beta2, w3, b3)

    wmap = _prep_weights(w1, g1, beta1, w2, b2, g2, beta2, w3, b3)

    import ml_dtypes
    xpad = np.zeros((NPAD, DIM), dtype=ml_dtypes.bfloat16)
    xpad[:N] = ten.astype(ml_dtypes.bfloat16)
    shards = xpad.reshape(N_CORES, ROWS_PER_CORE, DIM)

    nc = _get_nc()
    in_maps = [dict(wmap, x=np.ascontiguousarray(shards[c]))
               for c in range(N_CORES)]
    last_err = None
    for _attempt in range(3):
        try:
            res = run_bass_kernel_spmd(nc, in_maps, list(range(N_CORES))).results
            break
        except Exception as e:  # transient device-unrecoverable errors
            last_err = e
            import time as _time
            _time.sleep(10)
    else:
        raise last_err
    outp = np.concatenate([res[c]["out"] for c in range(N_CORES)], axis=0)
    return np.ascontiguousarray(outp[:N].astype(np.float32))


# revision 36
# speedup vs baseline: 1.4384x; 1.4384x over previous
"""Trainium2 Bass kernel for nn_EvMLP (segment_reduce EvNorm + invariant MLP).

Self-contained: hardcodes shapes/sharding. Accepts FULL inputs, returns FULL
output; shards the node dim N across 8 NeuronCores (pure data parallel).

v2 design (bf16 edition):
  - bf16 HBM I/O both directions (host converts; halves DMA traffic)
  - x10 loaded pre-transposed via DMA-transpose (HWDGE xbar) -> no PE
    transposes / ACT copies for the MLP input
  - single Newton-Raphson step for all rsqrts (quake seed on ACT)
  - W12 = w1p @ w2p host-side fusion: H1 only feeds the LN1 variance;
    H2 computed directly from x1T (kills the h1 PSUM->SBUF copy)
  - b3 bias added via rank-1 PE matmul into PSUM; out copy is plain ACT
  - bf16 matmuls (PE 2x), bf16 elementwise where DVE 2x modes apply
  - row order is linear "(b p)" so the DMA-transposed x10T and the
    eq-path tiles agree on row placement
"""
import sys

sys.path.insert(0, "/opt/trn_rl_repo")

import numpy as np

import concourse.bass as bass
import concourse.bacc as bacc
import concourse.tile as tile
from concourse import mybir
from concourse.bass_utils import run_bass_kernel_spmd

f32 = mybir.dt.float32
i32 = mybir.dt.int32
bf16 = mybir.dt.bfloat16

# ---------------------------------------------------------------- constants --
N = 100000
DIM = 592
N_INV = 128
N_EQ_CH = 112
N_EQ = 464
EPS = 1e-5
N_CORES = 8
BLOCKS_PER_CORE = 98                      # 98*128 = 12544 rows/core
ROWS_PER_CORE = BLOCKS_PER_CORE * 128
NPAD = N_CORES * ROWS_PER_CORE            # 100352
MACROS = [8] * 12 + [2]                   # blocks per macro-tile (sum 98)
CHUNK_BLOCKS = 4                          # rows per MLP chunk = 512
MAGIC = 0x5F3759DF
MAGICF = float(MAGIC)

# knob: eq column split for the square: [0:KA] ACT, [KA:KB] DVE, [KB:] GPSIMD
SQ_ACT_COLS = 192
SQ_DVE_COLS = 192

# segment groups: (n_channels, width, eq column offset, channel offset)
SEGS = [(64, 3, 0, 0), (32, 5, 192, 64), (16, 7, 352, 96)]

_EXPECTED_REP = np.concatenate(
    [np.repeat(np.arange(m) + off, 2 * l + 1)
     for l, (m, off) in enumerate([(128, 0), (64, 128), (32, 192), (16, 224)])]
)

# ------------------------------------------------------------- custom DVE op --
from concourse.dve_spec import Spec, Src0, Src1, C0, C1, C2, lower
from concourse.dve_uop import DveOpSpec
import concourse.dve_ops as dve_ops
from concourse.dve_ops import DveOp

# Newton rsqrt step: out = y*(C1 - C0*((v+C2)*y*y));  in0=v, in1=y
_nr_body = Src1 * (C1 - ((Src0 + C2) * (Src1 * Src1)) * C0)


def _nr_ref(in0, in1, s0, s1, imm2):
    y = in1.astype(np.float32)
    v = in0.astype(np.float32)
    return (y * (np.float32(s1) - ((v + np.float32(imm2)) * y * y) * np.float32(s0))
            ).astype(np.float32)


def _register(name, spec):
    if name in dve_ops._SUB_OPCODE_FOR_NAME:
        for op in dve_ops.OPS:
            if op.name == name:
                return op
    shas = {}
    row = 1 + len(dve_ops.OPS)
    for ver in ("v3", "v4"):
        s = DveOpSpec(name=name, opcode=row, uops=lower(spec, ver=ver), rd1_en=True)
        shas[ver] = s.sha(ver)
    op = DveOp(name, spec, subdim=False, uops_sha=shas)
    dve_ops.OPS.append(op)
    dve_ops._SUB_OPCODE_FOR_NAME[name] = row
    dve_ops.CUSTOM_DVE_SPECS[name] = spec
    return op


RSQRT_NR = _register("ANT_RSQRT_NR2", Spec(body=_nr_body, reference=_nr_ref))


def _make_mulsub1():
    from concourse.dve_spec import One
    return _register(
        "ANT_MUL_SUB1",
        Spec(
            body=(Src0 * Src1) - One,
            reference=lambda in0, in1, s0, s1, imm2: (
                in0.astype(np.float32) * in1 - np.float32(1.0)
            ).astype(np.float32),
        ),
    )


MUL_SUB1 = _make_mulsub1()


# ------------------------------------------------------------ kernel builder --
def _build_nc():
    nc = bacc.Bacc()

    x = nc.dram_tensor("x", [ROWS_PER_CORE, DIM], bf16, kind="ExternalInput")
    out = nc.dram_tensor("out", [ROWS_PER_CORE, DIM], bf16, kind="ExternalOutput")
    w1a_d = nc.dram_tensor("w1a", [128, 128], bf16, kind="ExternalInput")
    w1b_d = nc.dram_tensor("w1b", [112, 128], bf16, kind="ExternalInput")
    w12a_d = nc.dram_tensor("w12a", [128, 128], bf16, kind="ExternalInput")
    w12b_d = nc.dram_tensor("w12b", [112, 128], bf16, kind="ExternalInput")
    w3_d = nc.dram_tensor("w3p", [128, 128], bf16, kind="ExternalInput")
    cmat_d = nc.dram_tensor("cmat", [128, 128], bf16, kind="ExternalInput")
    onesd_d = nc.dram_tensor("onesd", [128, 128], bf16, kind="ExternalInput")
    ident_d = nc.dram_tensor("ident", [128, 128], bf16, kind="ExternalInput")
    b2_d = nc.dram_tensor("b2c", [128, 1], f32, kind="ExternalInput")
    ones1_d = nc.dram_tensor("ones1", [1, 128], bf16, kind="ExternalInput")
    b3r_d = nc.dram_tensor("b3rep", [1, 1024], bf16, kind="ExternalInput")

    # extra float consts used as activation bias (register like Bass.__init__)
    for _v in (MAGICF, float(EPS), 1.0):
        _t = nc.alloc_sbuf_tensor(f"const-f32-{_v}", [128, 1], f32)
        nc.gpsimd.memset(_t.ap(), _v)
        nc.const_aps.aps[(f32, _v)] = _t.ap()
    nc.all_engine_barrier()

    AF = mybir.ActivationFunctionType
    ALU = mybir.AluOpType
    AX = mybir.AxisListType

    from contextlib import ExitStack

    with tile.TileContext(nc) as tc:
        with ExitStack() as ctx:
            wpool = ctx.enter_context(tc.tile_pool(name="w", bufs=1))
            xpool = ctx.enter_context(tc.tile_pool(name="xp", bufs=4))
            opool = ctx.enter_context(tc.tile_pool(name="op", bufs=3))
            epool = ctx.enter_context(tc.tile_pool(name="ep", bufs=2))
            spool = ctx.enter_context(tc.tile_pool(name="sp", bufs=3))
            cpool = ctx.enter_context(tc.tile_pool(name="cp", bufs=3))
            ps_tp = ctx.enter_context(tc.tile_pool(name="ptp", bufs=2, space="PSUM"))
            ps_mm = ctx.enter_context(tc.tile_pool(name="pmm", bufs=4, space="PSUM"))
            ps_q = ctx.enter_context(tc.tile_pool(name="pq", bufs=2, space="PSUM"))

            def wtile(name, dram, shape, dtype):
                t = wpool.tile(shape, dtype, tag=name)
                nc.sync.dma_start(out=t, in_=dram[:, :])
                return t

            w1a = wtile("w1a", w1a_d, [128, 128], bf16)
            w1b = wtile("w1b", w1b_d, [112, 128], bf16)
            w12a = wtile("w12a", w12a_d, [128, 128], bf16)
            w12b = wtile("w12b", w12b_d, [112, 128], bf16)
            w3p = wtile("w3p", w3_d, [128, 128], bf16)
            cmat = wtile("cmat", cmat_d, [128, 128], bf16)
            onesd = wtile("onesd", onesd_d, [128, 128], bf16)
            ident = wtile("ident", ident_d, [128, 128], bf16)
            b2c = wtile("b2c", b2_d, [128, 1], f32)
            ones1 = wtile("ones1", ones1_d, [1, 128], bf16)
            b3rep = wtile("b3rep", b3r_d, [1, 1024], bf16)

            flat3 = lambda ap: ap.rearrange("p a b -> p (a b)")

            def mm512(out_ap, lhsT, rhs, start, stop, skip=False):
                # ISA caps the moving operand at 512 elements per matmul
                F = rhs.shape[-1]
                for f0 in range(0, F, 512):
                    f1 = min(f0 + 512, F)
                    nc.tensor.matmul(
                        out_ap[:, f0:f1], lhsT, rhs[:, f0:f1],
                        start=start, stop=stop, skip_group_check=skip,
                    )

            def issue_load(row0, nb):
                """Prefetch macro tiles (issued one iteration early)."""
                R_rows = nb * 128
                # linear row order: partition p, block b <-> row b*128 + p
                xev = x[row0 : row0 + R_rows, N_INV:DIM].rearrange(
                    "(b p) d -> p b d", p=128
                )
                x10v = x[row0 : row0 + R_rows, 0:N_INV]
                ov = out[row0 : row0 + R_rows, :].rearrange("(b p) d -> p b d", p=128)

                Xe = xpool.tile([128, nb, N_EQ], bf16, tag="Xe")
                nc.sync.dma_start(out=Xe, in_=xev)
                X10T = xpool.tile([128, R_rows], bf16, tag="X10T")
                nc.sync.dma_start(out=X10T, in_=x10v, transpose=True)
                return dict(nb=nb, ov=ov, Xe=Xe, X10T=X10T)

            def issue_eq(t):
                """eq chain through x11 (inputs prefetched last iteration)."""
                nb, Xe = t["nb"], t["Xe"]
                # ---- eq path (rows on partitions) ----
                eq2 = epool.tile([128, nb, N_EQ], bf16, tag="eq2")
                ka, kb = SQ_ACT_COLS, SQ_DVE_COLS
                nc.scalar.activation(
                    out=eq2[:, :, 0:ka], in_=Xe[:, :, 0:ka], func=AF.Square
                )
                if kb > ka:
                    nc.vector.tensor_tensor(
                        out=eq2[:, :, ka:kb], in0=Xe[:, :, ka:kb],
                        in1=Xe[:, :, ka:kb], op=ALU.mult,
                    )
                if N_EQ > kb:
                    nc.gpsimd.tensor_tensor(
                        out=eq2[:, :, kb:N_EQ], in0=Xe[:, :, kb:N_EQ],
                        in1=Xe[:, :, kb:N_EQ], op=ALU.mult,
                    )

                sumsq = spool.tile([128, nb, N_EQ_CH], f32, tag="sumsq")
                for (nch, w, eqoff, choff) in SEGS:
                    nc.vector.reduce_sum(
                        out=sumsq[:, :, choff : choff + nch],
                        in_=eq2[:, :, eqoff : eqoff + nch * w].rearrange(
                            "p b (c t) -> p b c t", t=w
                        ),
                        axis=AX.X,
                    )

                # s1 = sumsq + 1 (ACT); seed from its bits (ACT); 1 NR (DVE)
                s1 = spool.tile([128, nb, N_EQ_CH], f32, tag="s1")
                nc.scalar.activation(out=s1, in_=sumsq, func=AF.Identity, bias=1.0)
                seedb = spool.tile([128, nb, N_EQ_CH], i32, tag="seedb")
                nc.scalar.activation(
                    out=seedb, in_=s1.bitcast(i32), func=AF.Identity,
                    scale=-0.5, bias=MAGICF,
                )
                r = spool.tile([128, nb, N_EQ_CH], f32, tag="r")
                nc.vector._custom_dve(
                    RSQRT_NR, out=flat3(r), in0=flat3(s1),
                    in1=flat3(seedb.bitcast(f32)), s0=0.5, s1=1.5, imm2=0.0,
                )

                # x11 = s1 * r - 1  (= sqrt(s1) - 1)
                x11 = spool.tile([128, nb, N_EQ_CH], bf16, tag="x11")
                nc.vector._custom_dve(
                    MUL_SUB1, out=flat3(x11), in0=flat3(s1), in1=flat3(r),
                    s0=0.0, s1=0.0, imm2=0.0,
                )
                t["r"] = r
                t["x11"] = x11

            def issue_T(t):
                """Transposes of x11 + copy (inputs one iteration old)."""
                nb = t["nb"]
                x11 = t["x11"]
                chunks = []
                for cb0 in range(0, nb, CHUNK_BLOCKS):
                    cnb = min(CHUNK_BLOCKS, nb - cb0)
                    chunks.append((cb0, cnb, cnb * 128))
                t["chunks"] = chunks
                st = {}
                for ci, (cb0, cnb, R) in enumerate(chunks):
                    TPb = ps_tp.tile([N_EQ_CH, R], bf16, tag="tp")
                    for j in range(cnb):
                        nc.tensor.transpose(
                            TPb[:, j * 128 : (j + 1) * 128],
                            x11[:, cb0 + j, :], ident,
                        )
                    st[ci] = TPb
                xt = {}
                for ci, (cb0, cnb, R) in enumerate(chunks):
                    x11T = cpool.tile([N_EQ_CH, R], bf16, tag="x11T")
                    nc.scalar.activation(out=x11T, in_=st[ci], func=AF.Identity)
                    xt[ci] = x11T
                t["xt"] = xt

            def issue_L1(t):
                """x2 (GP), M1 + variance square."""
                nb = t["nb"]
                Xe, X10T, r = t["Xe"], t["X10T"], t["r"]
                chunks, xt = t["chunks"], t["xt"]
                O = opool.tile([128, nb, DIM], bf16, tag="O")
                t["O"] = O

                for (nch, w, eqoff, choff) in SEGS:
                    rbc = (
                        r[:, :, choff : choff + nch]
                        .unsqueeze(-1)
                        .broadcast_to((128, nb, nch, w))
                    )
                    nc.gpsimd.tensor_tensor(
                        out=O[:, :, N_INV + eqoff : N_INV + eqoff + nch * w].rearrange(
                            "p b (c t) -> p b c t", t=w
                        ),
                        in0=Xe[:, :, eqoff : eqoff + nch * w].rearrange(
                            "p b (c t) -> p b c t", t=w
                        ),
                        in1=rbc,
                        op=ALU.mult,
                    )

                sq1s = {}
                for ci, (cb0, cnb, R) in enumerate(chunks):
                    x10c = X10T[:, cb0 * 128 : cb0 * 128 + R]
                    H1 = ps_mm.tile([128, R], f32, tag="mm")
                    mm512(H1, w1a, x10c, True, False)
                    mm512(H1, w1b, xt[ci], False, True)
                    sq1 = cpool.tile([128, R], bf16, tag="sq1")
                    nc.scalar.activation(out=sq1, in_=H1, func=AF.Square)
                    sq1s[ci] = sq1
                t["sq1s"] = sq1s
                h2s = {}
                for ci, (cb0, cnb, R) in enumerate(chunks):
                    x10c = X10T[:, cb0 * 128 : cb0 * 128 + R]
                    H2 = ps_mm.tile([128, R], f32, tag="mm")
                    mm512(H2, w12a, x10c, True, False)
                    mm512(H2, w12b, xt[ci], False, True)
                    h2s[ci] = H2
                t["h2s"] = h2s

            def issue_L2(t):
                """LN1 stats+apply, silu, center, LN2 stats+apply."""
                nb = t["nb"]
                chunks, sq1s = t["chunks"], t["sq1s"]

                ln1 = {}
                for ci, (cb0, cnb, R) in enumerate(chunks):
                    Q1 = ps_q.tile([128, R], f32, tag="qq")
                    mm512(Q1, onesd, sq1s[ci], True, True)
                    sd1 = cpool.tile([128, R], i32, tag="sd1")
                    nc.scalar.activation(out=sd1, in_=Q1.bitcast(i32),
                                         func=AF.Identity, scale=-0.5, bias=MAGICF)
                    rstd1 = cpool.tile([128, R], f32, tag="rstd1")
                    nc.vector._custom_dve(
                        RSQRT_NR, out=rstd1, in0=Q1, in1=sd1.bitcast(f32),
                        s0=0.5, s1=1.5, imm2=float(EPS),
                    )
                    ln1[ci] = rstd1

                ln2 = {}
                h2s = t["h2s"]
                for ci, (cb0, cnb, R) in enumerate(chunks):
                    H2 = h2s[ci]
                    av = cpool.tile([128, R], bf16, tag="av")
                    nc.vector.tensor_mul(av, H2, ln1[ci])
                    a2 = cpool.tile([128, R], bf16, tag="a2")
                    nc.scalar.activation(out=a2, in_=av, func=AF.Silu, bias=b2c)
                    AC = ps_mm.tile([128, R], f32, tag="mm")
                    mm512(AC, cmat, a2, True, True)
                    sq2 = cpool.tile([128, R], bf16, tag="sq2")
                    nc.scalar.activation(out=sq2, in_=AC, func=AF.Square)
                    Q2 = ps_q.tile([128, R], f32, tag="qq")
                    mm512(Q2, onesd, sq2, True, True)
                    sd2 = cpool.tile([128, R], i32, tag="sd2")
                    nc.scalar.activation(out=sd2, in_=Q2.bitcast(i32),
                                         func=AF.Identity, scale=-0.5, bias=MAGICF)
                    rstd2 = cpool.tile([128, R], f32, tag="rstd2")
                    nc.vector._custom_dve(
                        RSQRT_NR, out=rstd2, in0=Q2, in1=sd2.bitcast(f32),
                        s0=0.5, s1=1.5, imm2=float(EPS),
                    )
                    hn2 = cpool.tile([128, R], bf16, tag="hn2")
                    nc.vector.tensor_mul(hn2, AC, rstd2)
                    ln2[ci] = hn2
                t["ln2"] = ln2

            def issue_L3(t):
                """M3 (+bias), out copy, store."""
                nb, ov, O = t["nb"], t["ov"], t["O"]
                chunks, ln2 = t["chunks"], t["ln2"]
                for ci, (cb0, cnb, R) in enumerate(chunks):
                    hn2 = ln2[ci]
                    H3n = ps_mm.tile([128, R], f32, tag="mm")
                    mm512(H3n, ones1, b3rep[:, 0:R], True, False, skip=True)
                    for j in range(cnb):
                        nc.tensor.matmul(
                            H3n[:, j * 128 : (j + 1) * 128],
                            hn2[:, j * 128 : (j + 1) * 128], w3p,
                            start=False, stop=True,
                            skip_group_check=True,
                        )
                    nc.scalar.activation(
                        out=O[:, cb0 : cb0 + cnb, 0:N_INV],
                        in_=H3n.rearrange("p (b j) -> p b j", j=128),
                        func=AF.Identity,
                    )
                nc.sync.dma_start(out=ov, in_=O)

            # ---- two-stage skewed software pipeline (v7b order) ----
            offs = []
            row0 = 0
            for nb in MACROS:
                offs.append((row0, nb))
                row0 += nb * 128
            states = {}
            nmac = len(offs)
            states[0] = issue_load(*offs[0])
            for i in range(nmac + 2):
                if i + 1 < nmac:
                    states[i + 1] = issue_load(*offs[i + 1])
                if i < nmac:
                    issue_eq(states[i])
                if 0 <= i - 1 < nmac:
                    issue_T(states[i - 1])
                    issue_L1(states[i - 1])
                if 0 <= i - 2 < nmac:
                    issue_L2(states[i - 2])
                    issue_L3(states[i - 2])
                    del states[i - 2]

    nc.finalize()
    return nc


_NC_CACHE = {}


def _get_nc():
    if "nc" not in _NC_CACHE:
        _NC_CACHE["nc"] = _build_nc()
    return _NC_CACHE["nc"]


# --------------------------------------------------------------- host driver --
def _bf16(a):
    import ml_dtypes
    return np.asarray(a).astype(ml_dtypes.bfloat16)


def _prep_weights(w1, g1, beta1, w2, b2, g2, beta2, w3, b3):
    C = np.eye(128, dtype=np.float64) - 1.0 / 128.0
    w1p = w1.astype(np.float64) @ C                       # [240,128]
    w2p = (g1.astype(np.float64)[:, None] * w2.astype(np.float64))
    b2c = beta1.astype(np.float64) @ w2.astype(np.float64) + b2.astype(np.float64)
    W12 = w1p @ w2p
    w3p = (g2.astype(np.float64)[:, None] * w3.astype(np.float64))
    b3c = beta2.astype(np.float64) @ w3.astype(np.float64) + b3.astype(np.float64)
    return {
        "w1a": _bf16(w1p[0:128]),
        "w1b": _bf16(w1p[128:240]),
        "w12a": _bf16(W12[0:128]),
        "w12b": _bf16(W12[128:240]),
        "w3p": _bf16(w3p),
        "cmat": _bf16(C),
        "onesd": _bf16(np.full((128, 128), 1.0 / 128.0)),
        "ident": _bf16(np.eye(128)),
        "b2c": b2c.astype(np.float32).reshape(128, 1),
        "ones1": _bf16(np.ones((1, 128))),
        "b3rep": _bf16(np.tile(b3c.reshape(1, 128), (1, 8))),
    }


def _np_reference(ten, w1, g1, beta1, w2, b2, g2, beta2, w3, b3):
    """Pure-numpy fallback (used only if rep_layout is unexpected)."""
    x10 = ten[:, :N_INV]
    eq = ten[:, N_INV:]
    sumsq = np.zeros((ten.shape[0], N_EQ_CH), np.float32)
    for (nch, w, eqoff, choff) in SEGS:
        sumsq[:, choff:choff + nch] = (
            (eq[:, eqoff:eqoff + nch * w].reshape(-1, nch, w) ** 2).sum(-1)
        )
    d = np.sqrt(sumsq + 1.0)
    x11 = d - 1.0
    x1 = np.concatenate([x10, x11], 1)
    seg = np.concatenate([np.repeat(np.arange(nch) + choff, w)
                          for (nch, w, eqoff, choff) in SEGS])
    x2 = eq / d[:, seg]

    def ln(h, g, b):
        mu = h.mean(-1, keepdims=True)
        var = h.var(-1, keepdims=True)
        return (h - mu) / np.sqrt(var + EPS) * g + b

    h = x1 @ w1
    h = ln(h, g1, beta1)
    h = h @ w2 + b2
    h = h * (1.0 / (1.0 + np.exp(-h)))
    h = ln(h, g2, beta2)
    h = h @ w3 + b3
    return np.concatenate([h, x2], 1).astype(np.float32)


def kernel(ten, rep_layout, w1, g1, beta1, w2, b2, g2, beta2, w3, b3):
    ten = np.asarray(ten, dtype=np.float32)
    args = [np.asarray(a) for a in (w1, g1, beta1, w2, b2, g2, beta2, w3, b3)]
    w1, g1, beta1, w2, b2, g2, beta2, w3, b3 = [a.astype(np.float32) for a in args]

    if not np.array_equal(np.asarray(rep_layout).astype(np.int64), _EXPECTED_REP):
        return _np_reference(ten, w1, g1, beta1, w2, b2, g2, beta2, w3, b3)

    wmap = _prep_weights(w1, g1, beta1, w2, b2, g2, beta2, w3, b3)

    import ml_dtypes
    xpad = np.zeros((NPAD, DIM), dtype=ml_dtypes.bfloat16)
    xpad[:N] = ten.astype(ml_dtypes.bfloat16)
    shards = xpad.reshape(N_CORES, ROWS_PER_CORE, DIM)

    nc = _get_nc()
    in_maps = [dict(wmap, x=np.ascontiguousarray(shards[c]))
               for c in range(N_CORES)]
    last_err = None
    for _attempt in range(3):
        try:
            res = run_bass_kernel_spmd(nc, in_maps, list(range(N_CORES))).results
            break
        except Exception as e:  # transient device-unrecoverable errors
            last_err = e
            import time as _time
            _time.sleep(10)
    else:
        raise last_err
    outp = np.concatenate([res[c]["out"] for c in range(N_CORES)], axis=0)
    return np.ascontiguousarray(outp[:N].astype(np.float32))
